# revision 15
# baseline (speedup 1.0000x reference)
"""Trainium2 Bass kernel for nn_Attention_88441966559243.

Attention with additive bias [B,N,N] and per-key bool mask, fp32.
  B=2, N=2048, QD=1024, HEADS=16, DIM_HEAD=64.

Sharding: 8 cores = (batch b = core//4) x (query slice q0 = (core%4)*512),
with Megatron-style tensor parallelism over heads for the K/V projections:
core r (within its batch group of 4) projects k for head-pairs 2r,2r+1 and
v for heads 4r..4r+3 only, then the group AllGathers the bf16 k-slabs and
v' blocks through DRAM bounce buffers.  This removes the 4x-redundant K/V
projection work each core used to do.

Key compaction: masked keys contribute exactly zero to the softmax
(exp(bias - 30000) underflows to 0, matching the reference's exp(-FMAX)),
so the host gathers only the unmasked keys per batch and pads to NKC
(multiple of 512).  With the given ~50% mask density this halves key-side
work and DMA.  Padded key slots get maskneg=-30000 so EB=0 and they
contribute nothing (numerator or denominator).

All inputs ride in bf16 (x, bias, weights); projections accumulate in fp32
PSUM; e/v'/k/q are bf16.  Measured rel err vs the fp32 reference ~6e-3.

Per-core stages:
  A: k-part -> kbounce, v'-part -> vbounce (collectives fire early),
     q projection (qT bf16), EB = exp(biasT + maskneg) on ACT.
  AllGather k (kfull [1024,NKC]) and v' (vfull [4*NKC,4*65]) per batch group.
  C: per head-pair hp: kst slab DMA'd from kfull; sim = kst^T q (paired
     K=64 matmuls); e = exp(sim*scale)*EB bf16 (ACT exp + DVE/GpSimd mul);
     av accumulates [v|1]^T e into PSUM (ones column gives the softmax
     denominator for free); normalize deferred into the next pair's loop.
  D: out = ot^T Wo + bo as K=128 bf16 matmuls.
"""
import sys
for _p in ("/opt/trn_rl_repo", "/root/.axon_site/_ro/trn_rl_repo"):
    if _p not in sys.path:
        sys.path.insert(0, _p)

import numpy as np

import concourse.bass as bass
import concourse.mybir as mybir
from concourse import bacc
from concourse.tile import TileContext
from concourse.bass_utils import run_bass_kernel_spmd

F = 1024          # feature dim (QD == INNER)
Q = 512           # queries per core
H = 16            # heads
D = 64            # head dim
DV = 65           # head dim + ones column
SCALE = D ** -0.5
MASK_NEG = -30000.0

FC = F // 128      # 8 feature chunks
HP = H // 2        # 8 head pairs

f32 = mybir.dt.float32
fr = mybir.dt.float32r
bf16 = mybir.dt.bfloat16
AF = mybir.ActivationFunctionType

AV_LAG = 3         # av matmuls trail the exp/mul producers by this many kc
RG = [[0, 1, 2, 3], [4, 5, 6, 7]]   # batch groups for the kv AllGather


def build_nc(niter: int = 1, nkc: int = 1024, use_tp: bool = True):
    KC = nkc // 128    # key chunks
    NB = nkc // 512    # key 512-blocks
    assert nkc % 512 == 0 and nkc >= 512
    # with tp: this core projects k for 2 head-pairs / v for 4 heads and the
    # batch group AllGathers; without: it computes all of k/v locally (no
    # collectives -- the robust fallback).
    KW = 256 if use_tp else 1024   # wk/wv input columns

    nc = bacc.Bacc(None, target_bir_lowering=False, num_devices=8)

    xT_in = nc.dram_tensor("xT_in", [F, Q + nkc], bf16, kind="ExternalInput")
    biasT_in = nc.dram_tensor("biasT_in", [nkc, Q], bf16,
                              kind="ExternalInput")
    maskneg_in = nc.dram_tensor("maskneg_in", [128, KC], f32,
                                kind="ExternalInput")
    wq_in = nc.dram_tensor("wq_in", [F, F], bf16, kind="ExternalInput")
    wk_in = nc.dram_tensor("wk_in", [F, KW], bf16, kind="ExternalInput")
    wv_in = nc.dram_tensor("wv_in", [F, KW], bf16, kind="ExternalInput")
    wo_in = nc.dram_tensor("wo_in", [F, F], bf16, kind="ExternalInput")
    bo_in = nc.dram_tensor("bo_in", [1, F], fr, kind="ExternalInput")
    out_t = nc.dram_tensor("out_t", [Q, F], f32, kind="ExternalOutput")

    with TileContext(nc) as tc:
        with (
            tc.tile_pool(name="const", bufs=1) as constp,
            tc.tile_pool(name="dram", bufs=1, space="DRAM") as dramp,
            tc.tile_pool(name="ps2", bufs=2, space="PSUM") as ps2p,   # [128,1024]
            tc.tile_pool(name="psk", bufs=2, space="PSUM") as pskp,   # [128,512]
            tc.tile_pool(name="psu", bufs=2, space="PSUM") as psup,   # [DV,512]
        ):
            ones_f = constp.tile([128, 128], f32)
            nc.vector.memset(ones_f[:, :], 1.0)
            ones_r = constp.tile([128, 128], fr)
            nc.scalar.copy(ones_r[:, :], ones_f[:, :])
            masksb = constp.tile([128, KC], f32)
            nc.sync.dma_start(masksb[:, :], maskneg_in[:, :])

            kfull = dramp.tile([8 * 128, nkc], bf16)    # all 8 pair slabs
            vfull = dramp.tile([4 * nkc, 4 * DV], bf16)  # all 4 quads
            if use_tp:
                kbounce = dramp.tile([256, nkc], bf16)   # my 2 k-pair slabs
                vbounce = dramp.tile([nkc, 4 * DV], bf16)  # my v' quad

            def body(_iv=None):
              with (
                  tc.tile_pool(name="oMp", bufs=1) as oMp,
                  tc.tile_pool(name="wop", bufs=1) as wop,
              ):
                # out^T head-pair tiles (bf16), written in-place by C's ot
                # DMAs; wo (bf16) loaded near the end of C.  Stacking heads
                # 2i/2i+1 on partitions 0:64/64:128 makes D K=128 full-rate.
                oM = [oMp.tile([128, Q], bf16, tag=f"oM{i}", name=f"oM{i}")
                      for i in range(H // 2)]
                wo = [wop.tile([128, F], bf16, tag=f"wo{i}", name=f"wo{i}")
                      for i in range(H // 2)]
                with (
                    tc.tile_pool(name="EBp", bufs=1) as EBp,
                    tc.tile_pool(name="qTp", bufs=1) as qTp,
                ):
                    EB = [EBp.tile([128, 2 * Q], bf16, tag=f"EB{i}",
                                   name=f"EB{i}") for i in range(KC)]
                    qT = [qTp.tile([128, Q], bf16, tag=f"qT{i}",
                                   name=f"qT{i}") for i in range(FC)]

                    # ---- A: per-core K/V quarter projections + q + EB ----
                    with (
                        tc.tile_pool(name="xTp", bufs=1) as xTp,
                        tc.tile_pool(name="wload", bufs=8) as wlp,
                        tc.tile_pool(name="wkv", bufs=1) as wkvp,
                        tc.tile_pool(name="kpsb", bufs=2) as kpsbp,
                        tc.tile_pool(name="vstg", bufs=6) as vstgp,
                    ):
                        xTc = [xTp.tile([128, 512], bf16, tag=f"xT{i}",
                                        name=f"xT{i}")
                               for i in range((1 + NB) * FC)]

                        def xt(fc, cb):
                            return xTc[fc * (1 + NB) + cb]

                        # loads: wk/wv/xk first (k/v parts fire the
                        # collectives), then wq/xq for A2.
                        wk_sb = wkvp.tile([128, FC * KW], bf16, tag="wk",
                                          name="wk")
                        nc.sync.dma_start(
                            wk_sb[:, :].rearrange("p (fc c) -> p fc c",
                                                  c=KW),
                            wk_in[0:F, :].rearrange("(fc p) c -> p fc c",
                                                    p=128))
                        wv_sb = wkvp.tile([128, FC * KW], bf16, tag="wv",
                                          name="wv")
                        nc.sync.dma_start(
                            wv_sb[:, :].rearrange("p (fc c) -> p fc c",
                                                  c=KW),
                            wv_in[0:F, :].rearrange("(fc p) c -> p fc c",
                                                    p=128))
                        for cb in range(1, 1 + NB):
                            for fc in range(FC):
                                nc.sync.dma_start(
                                    xt(fc, cb)[:, :],
                                    xT_in[fc * 128:(fc + 1) * 128,
                                          Q + (cb - 1) * 512:Q + cb * 512])
                        wq = [wlp.tile([128, F], bf16, tag="w", name="w")
                              for _ in range(FC)]
                        for fc in range(FC):
                            nc.sync.dma_start(
                                wq[fc][:, :],
                                wq_in[fc * 128:(fc + 1) * 128, :])
                        for fc in range(FC):
                            nc.sync.dma_start(
                                xt(fc, 0)[:, :],
                                xT_in[fc * 128:(fc + 1) * 128, 0:512])

                        # A-k: head-pair k slabs (2 with tp, all 8 without)
                        for pair in range(2 if use_tp else 8):
                            kp_sb = kpsbp.tile([128, nkc], bf16,
                                               name="kpsb")
                            for nb in range(NB):
                                ps = pskp.tile([128, 512], f32, name="psk")
                                for fc in range(FC):
                                    nc.tensor.matmul(
                                        ps[:, :],
                                        wk_sb[:, :].rearrange(
                                            "p (fc c) -> p fc c",
                                            c=KW)[:, fc,
                                                  pair * 128:
                                                  (pair + 1) * 128],
                                        xt(fc, 1 + nb)[:, :],
                                        start=(fc == 0),
                                        stop=(fc == FC - 1))
                                nc.vector.tensor_copy(
                                    kp_sb[:, nb * 512:(nb + 1) * 512],
                                    ps[:, :])
                            nc.sync.dma_start(
                                (kbounce if use_tp else kfull)[
                                    pair * 128:(pair + 1) * 128, :],
                                kp_sb[:, :])

                        # A-v: v' quads (mine with tp, all 4 without)
                        for hq in range(1 if use_tp else 4):
                            for kc in range(KC):
                                vst = vstgp.tile([128, 4 * DV], bf16,
                                                 name="vst")
                                nc.vector.memset(
                                    vst[:, :].rearrange(
                                        "p (h x) -> p h x",
                                        x=DV)[:, :, D:DV], 1.0)
                                ps = ps2p.tile([128, 1024], f32,
                                               name="ps2")
                                for fc in range(FC):
                                    nc.tensor.matmul(
                                        ps[:, 0:256],
                                        xt(fc, 1 + kc // 4)[
                                            :, (kc % 4) * 128:
                                            (kc % 4 + 1) * 128],
                                        wv_sb[:, fc * KW + hq * 256:
                                              fc * KW + (hq + 1) * 256],
                                        start=(fc == 0),
                                        stop=(fc == FC - 1))
                                nc.vector.tensor_copy(
                                    vst[:, :].rearrange(
                                        "p (h x) -> p h x",
                                        x=DV)[:, :, 0:D],
                                    ps[:, 0:256].rearrange(
                                        "p (h d) -> p h d", d=D))
                                if use_tp:
                                    nc.sync.dma_start(
                                        vbounce[kc * 128:(kc + 1) * 128,
                                                :],
                                        vst[:, :])
                                else:
                                    nc.sync.dma_start(
                                        vfull[hq * nkc + kc * 128:
                                              hq * nkc + (kc + 1) * 128,
                                              :],
                                        vst[:, :])

                        # A2: qT[m] = Wq[:,m]^T @ xT[:, 0:Q]  (unscaled;
                        # SCALE rides on the exp activation's scale)
                        for m in range(FC):
                            ps = pskp.tile([128, 512], f32, name="psk")
                            for fc in range(FC):
                                nc.tensor.matmul(
                                    ps[:, :],
                                    wq[fc][:, m * 128:(m + 1) * 128],
                                    xt(fc, 0)[:, :],
                                    start=(fc == 0), stop=(fc == FC - 1))
                            nc.vector.tensor_copy(qT[m][:, :], ps[:, :])

                        # B: EB = exp(biasT + maskneg); biasT loads ride
                        # the ACT hwdge queue
                        with tc.tile_pool(name="bT", bufs=4) as bTp:
                            for kc in range(KC):
                                bT = bTp.tile([128, Q], bf16, name="bT")
                                nc.scalar.dma_start(
                                    bT[:, :],
                                    biasT_in[kc * 128:(kc + 1) * 128, :])
                                nc.scalar.activation(
                                    EB[kc][:, 0:Q], bT[:, :], AF.Exp,
                                    bias=masksb[:, kc:kc + 1], scale=1.0)
                                nc.vector.tensor_copy(EB[kc][:, Q:2 * Q],
                                                      EB[kc][:, 0:Q])

                    # ---- kv AllGather across the batch group ----
                    if use_tp:
                        nc.gpsimd.collective_compute(
                            "AllGather", mybir.AluOpType.bypass,
                            replica_groups=RG,
                            ins=[kbounce[:, :].opt()],
                            outs=[kfull[:, :].opt()])
                        nc.gpsimd.collective_compute(
                            "AllGather", mybir.AluOpType.bypass,
                            replica_groups=RG,
                            ins=[vbounce[:, :].opt()],
                            outs=[vfull[:, :].opt()])

                    # ---- C: attention over head pairs ----
                    with (
                        tc.tile_pool(name="kst", bufs=2) as kstp,
                        tc.tile_pool(name="vph", bufs=2) as vphp,
                        tc.tile_pool(name="eraw", bufs=3) as erawp,
                        tc.tile_pool(name="et", bufs=9) as ep,
                        tc.tile_pool(name="dsb", bufs=1) as dsbp,
                        tc.tile_pool(name="rrep", bufs=1) as rrepp,
                        tc.tile_pool(name="otst", bufs=2) as otstp,
                    ):
                        def load_kst(hp):
                            kst = kstp.tile([128, nkc], bf16, name="kst")
                            nc.sync.dma_start(
                                kst[:, :],
                                kfull[hp * 128:(hp + 1) * 128, :])
                            return kst

                        def load_vquad(hq):
                            vph = vphp.tile([128, KC * 4 * DV], bf16,
                                            name="vph")
                            nc.sync.dma_start(
                                vph[:, :].rearrange(
                                    "p (kc d) -> p kc d", d=4 * DV),
                                vfull[hq * nkc:(hq + 1) * nkc, :]
                                .rearrange("(kc p) d -> p kc d", p=128))
                            return vph

                        def emit_denoms(hp0, psU0):
                            """Softmax normalize + oM write for pair hp0;
                            deferred into the next pair's kc loop so the
                            PE sim stream isn't interrupted."""
                            for sub in range(2):
                                Dsb = dsbp.tile([DV, 512], fr, name="Dsb")
                                nc.vector.tensor_copy(
                                    Dsb[64:65, :], psU0[sub][64:65, :])
                                psR = ps2p.tile([128, 1024], f32,
                                                name="ps2")
                                nc.tensor.matmul(psR[0:64, 0:512],
                                                 ones_r[64:65, 0:64],
                                                 Dsb[64:65, :],
                                                 start=True, stop=True)
                                rrep = rrepp.tile([64, 512], f32,
                                                  name="rrep")
                                nc.vector.reciprocal_approx_fast(
                                    out=rrep[:, :], in_=psR[0:64, 0:512])
                                ot = otstp.tile([64, Q], bf16, name="ot")
                                nc.vector.tensor_mul(ot[:, :],
                                                     psU0[sub][0:64, :],
                                                     rrep[:, :])
                                nc.sync.dma_start(
                                    oM[hp0][sub * 64:(sub + 1) * 64, :],
                                    ot[:, :])

                        kst_cur = load_kst(0)
                        vph_cur = load_vquad(0)
                        prev = None

                        for hp in range(HP):
                            if hp == 6:
                                for i in range(H // 2):
                                    nc.sync.dma_start(
                                        wo[i][:, :],
                                        wo_in[i * 128:(i + 1) * 128, :])
                            if hp % 2 == 1 and hp + 1 < HP:
                                vph_next = load_vquad((hp + 1) // 2)
                            kst_next = (load_kst(hp + 1)
                                        if hp + 1 < HP else None)
                            psU = [psup.tile([DV, 512], f32, name="psu")
                                   for _ in range(2)]
                            pending = []

                            def drain_av(upto):
                                while pending and pending[0][0] <= upto:
                                    kc0, eT = pending.pop(0)
                                    for sub in range(2):
                                        hq_off = (2 * hp + sub) % 4
                                        nc.tensor.matmul(
                                            psU[sub][:, :],
                                            vph_cur[:,
                                                    kc0 * 4 * DV
                                                    + hq_off * DV:
                                                    kc0 * 4 * DV
                                                    + (hq_off + 1) * DV],
                                            eT[:, sub * Q:(sub + 1) * Q],
                                            start=(kc0 == 0),
                                            stop=(kc0 == KC - 1))

                            for kc in range(KC):
                                ps = ps2p.tile([128, 1024], f32,
                                               name="ps2")
                                for sub in range(2):
                                    po = sub * 64
                                    nc.tensor.matmul(
                                        ps[:, sub * Q:(sub + 1) * Q],
                                        kst_cur[po:po + 64,
                                                kc * 128:(kc + 1) * 128],
                                        qT[hp][po:po + 64, :],
                                        start=True, stop=True)
                                if kc == 1 and prev is not None:
                                    emit_denoms(*prev)
                                eRaw = erawp.tile([128, 1024], bf16,
                                                  name="eRaw")
                                nc.scalar.activation(
                                    eRaw[:, :], ps[:, :], AF.Exp,
                                    scale=SCALE)
                                eT = ep.tile([128, 1024], bf16,
                                             name="eT")
                                nc.vector.tensor_mul(eT[:, :], eRaw[:, :],
                                                     EB[kc][:, :])
                                pending.append((kc, eT))
                                # longer lag on hp0 gives the v' AllGather
                                # time to land before the first av needs it
                                drain_av(kc - (6 if hp == 0 else AV_LAG))
                            drain_av(KC)
                            prev = (hp, psU)
                            kst_cur = kst_next
                            if hp % 2 == 1 and hp + 1 < HP:
                                vph_cur = vph_next
                        emit_denoms(*prev)

                # ======== stage D ========
                with (
                    tc.tile_pool(name="fin", bufs=3) as finp,
                    tc.tile_pool(name="bop", bufs=1) as bop,
                ):
                    bo_sb = bop.tile([1, F], fr, name="bo_sb")
                    nc.sync.dma_start(bo_sb[:, :], bo_in[:, :])
                    bo_rep = bop.tile([128, F], f32, name="bo_rep")
                    for nb2 in range(2):
                        ps = pskp.tile([128, 512], f32, name="psk")
                        nc.tensor.matmul(ps[:, :], ones_r[0:1, 0:128],
                                         bo_sb[0:1, nb2 * 512:(nb2 + 1) * 512],
                                         start=True, stop=True)
                        nc.vector.tensor_copy(
                            bo_rep[:, nb2 * 512:(nb2 + 1) * 512], ps[:, :])
                    for mc in range(4):
                        for nb2 in range(2):
                            psF = pskp.tile([128, 512], f32, name="psk")
                            for h in range(H // 2):
                                nc.tensor.matmul(
                                    psF[:, :],
                                    oM[h][:, mc * 128:(mc + 1) * 128],
                                    wo[h][:, nb2 * 512:(nb2 + 1) * 512],
                                    start=(h == 0), stop=(h == H // 2 - 1))
                            fin = finp.tile([128, 512], f32, name="fin")
                            nc.vector.tensor_add(
                                fin[:, :], psF[:, :],
                                bo_rep[:, nb2 * 512:(nb2 + 1) * 512])
                            nc.scalar.dma_start(
                                out_t[mc * 128:(mc + 1) * 128,
                                      nb2 * 512:(nb2 + 1) * 512],
                                fin[:, :])

            if niter == 1:
                body()
            else:
                with tc.For_i(0, niter, 1) as iv:
                    body(iv)

    nc.finalize()
    return nc


_nc_cache = {}


def _get_nc(niter=1, nkc=1024, use_tp=True):
    key = (niter, nkc, use_tp)
    if key not in _nc_cache:
        _nc_cache[key] = build_nc(niter, nkc, use_tp)
    return _nc_cache[key]


def nkc_for_mask(mask):
    mask = np.asarray(mask)
    nk = int(mask.sum(axis=1).max())
    return max(512, 512 * ((nk + 511) // 512))


def make_in_maps(x, bias, mask, Wq, Wkv, Wo, bo, nkc=None, use_tp=True):
    x = np.asarray(x, dtype=np.float32)
    bias = np.asarray(bias, dtype=np.float32)
    mask = np.asarray(mask)
    if nkc is None:
        nkc = nkc_for_mask(mask)
    KC = nkc // 128
    bfnp = mybir.dt.np(bf16)
    in_maps = []
    wq_h = np.ascontiguousarray(np.asarray(Wq, dtype=np.float32).astype(bfnp))
    wkv_h = np.asarray(Wkv, dtype=np.float32).astype(bfnp)
    wo_h = np.ascontiguousarray(np.asarray(Wo).astype(bfnp))
    bo_h = np.ascontiguousarray(
        np.asarray(bo, dtype=np.float32).reshape(1, F))
    for c in range(8):
        b, r = c // 4, c % 4
        q0 = r * Q
        keys = np.flatnonzero(mask[b])
        nk_eff = len(keys)
        assert nk_eff <= nkc, (nk_eff, nkc)
        xT = np.zeros((F, Q + nkc), dtype=bfnp)
        xT[:, 0:Q] = x[b, q0:q0 + Q].T.astype(bfnp)
        xT[:, Q:Q + nk_eff] = x[b, keys].T.astype(bfnp)
        biasT = np.zeros((nkc, Q), dtype=bfnp)
        biasT[0:nk_eff] = bias[b, q0:q0 + Q][:, keys].T.astype(bfnp)
        maskneg = np.full(nkc, MASK_NEG, dtype=np.float32)
        maskneg[0:nk_eff] = 0.0
        if use_tp:
            wk_h = np.ascontiguousarray(wkv_h[:, 256 * r:256 * (r + 1)])
            wv_h = np.ascontiguousarray(
                wkv_h[:, F + 256 * r:F + 256 * (r + 1)])
        else:
            wk_h = np.ascontiguousarray(wkv_h[:, 0:F])
            wv_h = np.ascontiguousarray(wkv_h[:, F:2 * F])
        in_maps.append({
            "xT_in": np.ascontiguousarray(xT),
            "biasT_in": np.ascontiguousarray(biasT),
            "maskneg_in": np.ascontiguousarray(maskneg.reshape(KC, 128).T),
            "wq_in": wq_h,
            "wk_in": wk_h,
            "wv_in": wv_h,
            "wo_in": wo_h,
            "bo_in": bo_h,
        })
    return in_maps


class _CachedRunner:
    """Jit the NEFF-backed executable once; repeat kernel() calls then skip
    the ~40s relower/recompile and run in ~0.1s."""

    def __init__(self, nc, n_cores=8):
        import jax
        from jax.sharding import Mesh, PartitionSpec
        from jax.experimental.shard_map import shard_map
        from concourse.bass2jax import (_bass_exec_p, install_neuronx_cc_hook,
                                        partition_id_tensor)
        install_neuronx_cc_hook()
        self.jax = jax
        self.n_cores = n_cores
        pname = nc.partition_id_tensor.name if nc.partition_id_tensor else None
        in_names, out_names, out_avals, zeros = [], [], [], []
        for alloc in nc.m.functions[0].allocations:
            if not isinstance(alloc, mybir.MemoryLocationSet):
                continue
            name = alloc.memorylocations[0].name
            if alloc.kind == "ExternalInput":
                if name != pname:
                    in_names.append(name)
            elif alloc.kind == "ExternalOutput":
                out_names.append(name)
                shape = tuple(alloc.tensor_shape)
                dt_np = mybir.dt.np(alloc.dtype)
                out_avals.append(jax.core.ShapedArray(shape, dt_np))
                zeros.append(np.zeros(shape, dt_np))
        self.in_names, self.out_names = in_names, out_names
        self.out_avals, self.zeros = out_avals, zeros
        all_names = in_names + out_names + ([pname] if pname else [])

        def _body(*args):
            ops = list(args)
            if pname is not None:
                ops.append(partition_id_tensor())
            return tuple(_bass_exec_p.bind(
                *ops, out_avals=tuple(out_avals), in_names=tuple(all_names),
                out_names=tuple(out_names), lowering_input_output_aliases=(),
                sim_require_finite=True, sim_require_nnan=True, nc=nc))

        mesh = Mesh(np.asarray(jax.devices()[:n_cores]), ("core",))
        spec_in = (PartitionSpec("core"),) * (len(in_names) + len(out_names))
        spec_out = (PartitionSpec("core"),) * len(out_names)
        self.fn = jax.jit(shard_map(_body, mesh=mesh, in_specs=spec_in,
                                    out_specs=spec_out, check_rep=False),
                          keep_unused=True)

    def run(self, in_maps):
        n = self.n_cores
        args = [np.concatenate([np.asarray(in_maps[c][k]) for c in range(n)], axis=0)
                for k in self.in_names]
        args += [np.zeros((n * z.shape[0], *z.shape[1:]), z.dtype)
                 for z in self.zeros]
        outs = self.fn(*args)
        self.jax.block_until_ready(outs)
        return [{k: np.asarray(outs[i]).reshape(n, *self.out_avals[i].shape)[c]
                 for i, k in enumerate(self.out_names)} for c in range(n)]


_runner_cache = {}


def kernel(x, bias, mask, Wq, Wkv, Wo, bo):
    nkc = nkc_for_mask(mask)
    try:
        key = (nkc, True)
        if key not in _runner_cache:
            _runner_cache[key] = _CachedRunner(_get_nc(1, nkc, True))
        in_maps = make_in_maps(x, bias, mask, Wq, Wkv, Wo, bo, nkc=nkc,
                               use_tp=True)
        results = _runner_cache[key].run(in_maps)
    except Exception:
        # robust fallback: no-collective variant (k/v computed locally)
        _runner_cache.pop((nkc, True), None)
        in_maps = make_in_maps(x, bias, mask, Wq, Wkv, Wo, bo, nkc=nkc,
                               use_tp=False)
        res = run_bass_kernel_spmd(_get_nc(1, nkc, False), in_maps,
                                   core_ids=list(range(8)))
        results = res.results
    out = np.empty((2, 2048, F), dtype=np.float32)
    for c in range(8):
        b, qi = c // 4, c % 4
        out[b, qi * Q:(qi + 1) * Q] = results[c]["out_t"]
    return out


# revision 20
# speedup vs baseline: 5.8901x; 5.8901x over previous
"""Trainium2 Bass kernel for nn_Attention_88441966559243.

Attention with additive bias [B,N,N] and per-key bool mask, fp32.
  B=2, N=2048, QD=1024, HEADS=16, DIM_HEAD=64.

Sharding: 8 cores = (batch b = core//4) x (query slice q0 = (core%4)*512),
with Megatron-style tensor parallelism over heads for the K/V projections:
core r (within its batch group of 4) projects k for head-pairs 2r,2r+1 and
v for heads 4r..4r+3 only, then the group AllGathers the bf16 k-slabs and
v' blocks through DRAM bounce buffers.  This removes the 4x-redundant K/V
projection work each core used to do.

Key compaction: masked keys contribute exactly zero to the softmax
(exp(bias - 30000) underflows to 0, matching the reference's exp(-FMAX)),
so the host gathers only the unmasked keys per batch and pads to NKC
(multiple of 512).  With the given ~50% mask density this halves key-side
work and DMA.  Padded key slots get maskneg=-30000 so EB=0 and they
contribute nothing (numerator or denominator).

All inputs ride in bf16 (x, bias, weights); projections accumulate in fp32
PSUM; e/v'/k/q are bf16.  Measured rel err vs the fp32 reference ~6e-3.

Per-core stages:
  A: k-part -> kbounce, v'-part -> vbounce (collectives fire early),
     q projection (qT bf16), EB = exp(biasT + maskneg) on ACT.
  AllGather k (kfull [1024,NKC]) and v' (vfull [4*NKC,4*65]) per batch group.
  C: per head-pair hp: kst slab DMA'd from kfull; sim = kst^T q (paired
     K=64 matmuls); e = exp(sim*scale)*EB bf16 (ACT exp + DVE/GpSimd mul);
     av accumulates [v|1]^T e into PSUM (ones column gives the softmax
     denominator for free); normalize deferred into the next pair's loop.
  D: out = ot^T Wo + bo as K=128 bf16 matmuls.
"""
import sys
for _p in ("/opt/trn_rl_repo", "/root/.axon_site/_ro/trn_rl_repo"):
    if _p not in sys.path:
        sys.path.insert(0, _p)

import numpy as np

import concourse.bass as bass
import concourse.mybir as mybir
from concourse import bacc
from concourse.tile import TileContext
from concourse.bass_utils import run_bass_kernel_spmd

F = 1024          # feature dim (QD == INNER)
Q = 512           # queries per core
H = 16            # heads
D = 64            # head dim
DV = 65           # head dim + ones column
SCALE = D ** -0.5
MASK_NEG = -30000.0

FC = F // 128      # 8 feature chunks
HP = H // 2        # 8 head pairs

f32 = mybir.dt.float32
fr = mybir.dt.float32r
bf16 = mybir.dt.bfloat16
AF = mybir.ActivationFunctionType

AV_LAG = 3         # av matmuls trail the exp/mul producers by this many kc
RG = [[0, 1, 2, 3], [4, 5, 6, 7]]   # batch groups for the kv AllGather


def build_nc(niter: int = 1, nkc: int = 1024, mode: str = "kv"):
    KC = nkc // 128    # key chunks
    NB = nkc // 512    # key 512-blocks
    assert nkc % 512 == 0 and nkc >= 512
    assert mode in ("kv", "v", "none")
    # mode "kv": this core projects k for 2 head-pairs and v for 4 heads,
    #   and the batch group AllGathers both (2 collectives).
    # mode "v": k computed fully locally; only the v' quad is AllGathered
    #   (1 collective, hidden behind a deep hp0 av lag).
    # mode "none": all local, no collectives (robust fallback).
    tp_k = mode == "kv"
    tp_v = mode in ("kv", "v")
    KWK = 256 if tp_k else 1024   # wk input columns
    KWV = 256 if tp_v else 1024   # wv input columns

    nc = bacc.Bacc(None, target_bir_lowering=False, num_devices=8)

    xT_in = nc.dram_tensor("xT_in", [F, Q + nkc], bf16, kind="ExternalInput")
    biasT_in = nc.dram_tensor("biasT_in", [nkc, Q], bf16,
                              kind="ExternalInput")
    maskneg_in = nc.dram_tensor("maskneg_in", [128, KC], f32,
                                kind="ExternalInput")
    wq_in = nc.dram_tensor("wq_in", [F, F], bf16, kind="ExternalInput")
    wk_in = nc.dram_tensor("wk_in", [F, KWK], bf16, kind="ExternalInput")
    wv_in = nc.dram_tensor("wv_in", [F, KWV], bf16, kind="ExternalInput")
    wo_in = nc.dram_tensor("wo_in", [F, F], bf16, kind="ExternalInput")
    bo_in = nc.dram_tensor("bo_in", [1, F], fr, kind="ExternalInput")
    out_t = nc.dram_tensor("out_t", [Q, F], f32, kind="ExternalOutput")

    with TileContext(nc) as tc:
        with (
            tc.tile_pool(name="const", bufs=1) as constp,
            tc.tile_pool(name="dram", bufs=1, space="DRAM") as dramp,
            tc.tile_pool(name="ps2", bufs=2, space="PSUM") as ps2p,   # [128,1024]
            tc.tile_pool(name="psk", bufs=2, space="PSUM") as pskp,   # [128,512]
            tc.tile_pool(name="psu", bufs=2, space="PSUM") as psup,   # [DV,512]
        ):
            ones_f = constp.tile([128, 128], f32)
            nc.vector.memset(ones_f[:, :], 1.0)
            ones_r = constp.tile([128, 128], fr)
            nc.scalar.copy(ones_r[:, :], ones_f[:, :])
            masksb = constp.tile([128, KC], f32)
            nc.sync.dma_start(masksb[:, :], maskneg_in[:, :])

            kfull = dramp.tile([8 * 128, nkc], bf16)    # all 8 pair slabs
            vfull = dramp.tile([4 * nkc, 4 * DV], bf16)  # all 4 quads
            if tp_k:
                kbounce = dramp.tile([256, nkc], bf16)   # my 2 k-pair slabs
            if tp_v:
                vbounce = dramp.tile([nkc, 4 * DV], bf16)  # my v' quad

            def body(_iv=None):
              with (
                  tc.tile_pool(name="oMp", bufs=1) as oMp,
                  tc.tile_pool(name="wop", bufs=1) as wop,
              ):
                # out^T head-pair tiles (bf16), written in-place by C's ot
                # DMAs; wo (bf16) loaded near the end of C.  Stacking heads
                # 2i/2i+1 on partitions 0:64/64:128 makes D K=128 full-rate.
                oM = [oMp.tile([128, Q], bf16, tag=f"oM{i}", name=f"oM{i}")
                      for i in range(H // 2)]
                wo = [wop.tile([128, F], bf16, tag=f"wo{i}", name=f"wo{i}")
                      for i in range(H // 2)]
                with (
                    tc.tile_pool(name="EBp", bufs=1) as EBp,
                    tc.tile_pool(name="qTp", bufs=1) as qTp,
                ):
                    EB = [EBp.tile([128, 2 * Q], bf16, tag=f"EB{i}",
                                   name=f"EB{i}") for i in range(KC)]
                    qT = [qTp.tile([128, Q], bf16, tag=f"qT{i}",
                                   name=f"qT{i}") for i in range(FC)]

                    # ---- A: per-core K/V quarter projections + q + EB ----
                    with (
                        tc.tile_pool(name="xTp", bufs=1) as xTp,
                        tc.tile_pool(name="wload", bufs=8) as wlp,
                        tc.tile_pool(name="wkv", bufs=1) as wkvp,
                        tc.tile_pool(name="kpsb", bufs=2) as kpsbp,
                        tc.tile_pool(name="vstg", bufs=6) as vstgp,
                    ):
                        xTc = [xTp.tile([128, 512], bf16, tag=f"xT{i}",
                                        name=f"xT{i}")
                               for i in range((1 + NB) * FC)]

                        def xt(fc, cb):
                            return xTc[fc * (1 + NB) + cb]

                        # PE p-state warmup: ~3-4us of junk matmuls on the
                        # ones tile ramp the tensor engine to full clock
                        # while the input DMAs land, so stage A's real
                        # matmuls start at 2.4GHz instead of 0.65-1.2GHz.
                        ps_w = pskp.tile([128, 512], f32, name="psk")
                        for _w in range(24):
                            nc.tensor.matmul(
                                ps_w[:, 0:128], ones_r[:, :], ones_r[:, :],
                                start=(_w == 0), stop=(_w == 23))
                        warm_sink = constp.tile([1, 128], f32)
                        nc.vector.tensor_copy(warm_sink[:, :],
                                              ps_w[0:1, 0:128])

                        # loads: wk/wv/xk first (k/v parts fire the
                        # collectives), then wq/xq for A2.
                        wk_sb = wkvp.tile([128, FC * KWK], bf16, tag="wk",
                                          name="wk")
                        nc.sync.dma_start(
                            wk_sb[:, :].rearrange("p (fc c) -> p fc c",
                                                  c=KWK),
                            wk_in[0:F, :].rearrange("(fc p) c -> p fc c",
                                                    p=128))
                        wv_sb = wkvp.tile([128, FC * KWV], bf16, tag="wv",
                                          name="wv")
                        nc.sync.dma_start(
                            wv_sb[:, :].rearrange("p (fc c) -> p fc c",
                                                  c=KWV),
                            wv_in[0:F, :].rearrange("(fc p) c -> p fc c",
                                                    p=128))
                        for cb in range(1, 1 + NB):
                            for fc in range(FC):
                                nc.sync.dma_start(
                                    xt(fc, cb)[:, :],
                                    xT_in[fc * 128:(fc + 1) * 128,
                                          Q + (cb - 1) * 512:Q + cb * 512])
                        wq = [wlp.tile([128, F], bf16, tag="w", name="w")
                              for _ in range(FC)]
                        for fc in range(FC):
                            nc.sync.dma_start(
                                wq[fc][:, :],
                                wq_in[fc * 128:(fc + 1) * 128, :])
                        for fc in range(FC):
                            nc.sync.dma_start(
                                xt(fc, 0)[:, :],
                                xT_in[fc * 128:(fc + 1) * 128, 0:512])

                        # A-k: head-pair k slabs (2 with tp-k, else all 8)
                        for pair in range(2 if tp_k else 8):
                            kp_sb = kpsbp.tile([128, nkc], bf16,
                                               name="kpsb")
                            for nb in range(NB):
                                ps = pskp.tile([128, 512], f32, name="psk")
                                for fc in range(FC):
                                    nc.tensor.matmul(
                                        ps[:, :],
                                        wk_sb[:, :].rearrange(
                                            "p (fc c) -> p fc c",
                                            c=KWK)[:, fc,
                                                  pair * 128:
                                                  (pair + 1) * 128],
                                        xt(fc, 1 + nb)[:, :],
                                        start=(fc == 0),
                                        stop=(fc == FC - 1))
                                nc.vector.tensor_copy(
                                    kp_sb[:, nb * 512:(nb + 1) * 512],
                                    ps[:, :])
                            nc.sync.dma_start(
                                (kbounce if tp_k else kfull)[
                                    pair * 128:(pair + 1) * 128, :],
                                kp_sb[:, :])

                        # A-v: v' quads (mine with tp-v, else all 4)
                        for hq in range(1 if tp_v else 4):
                            for kc in range(KC):
                                vst = vstgp.tile([128, 4 * DV], bf16,
                                                 name="vst")
                                nc.vector.memset(
                                    vst[:, :].rearrange(
                                        "p (h x) -> p h x",
                                        x=DV)[:, :, D:DV], 1.0)
                                ps = ps2p.tile([128, 1024], f32,
                                               name="ps2")
                                for fc in range(FC):
                                    nc.tensor.matmul(
                                        ps[:, 0:256],
                                        xt(fc, 1 + kc // 4)[
                                            :, (kc % 4) * 128:
                                            (kc % 4 + 1) * 128],
                                        wv_sb[:, fc * KWV + hq * 256:
                                              fc * KWV + (hq + 1) * 256],
                                        start=(fc == 0),
                                        stop=(fc == FC - 1))
                                nc.vector.tensor_copy(
                                    vst[:, :].rearrange(
                                        "p (h x) -> p h x",
                                        x=DV)[:, :, 0:D],
                                    ps[:, 0:256].rearrange(
                                        "p (h d) -> p h d", d=D))
                                if tp_v:
                                    nc.sync.dma_start(
                                        vbounce[kc * 128:(kc + 1) * 128,
                                                :],
                                        vst[:, :])
                                else:
                                    nc.sync.dma_start(
                                        vfull[hq * nkc + kc * 128:
                                              hq * nkc + (kc + 1) * 128,
                                              :],
                                        vst[:, :])

                        # A2: qT[m] = Wq[:,m]^T @ xT[:, 0:Q]  (unscaled;
                        # SCALE rides on the exp activation's scale)
                        for m in range(FC):
                            ps = pskp.tile([128, 512], f32, name="psk")
                            for fc in range(FC):
                                nc.tensor.matmul(
                                    ps[:, :],
                                    wq[fc][:, m * 128:(m + 1) * 128],
                                    xt(fc, 0)[:, :],
                                    start=(fc == 0), stop=(fc == FC - 1))
                            nc.vector.tensor_copy(qT[m][:, :], ps[:, :])

                        # B: EB = exp(biasT + maskneg); biasT loads ride
                        # the ACT hwdge queue
                        with tc.tile_pool(name="bT", bufs=4) as bTp:
                            for kc in range(KC):
                                bT = bTp.tile([128, Q], bf16, name="bT")
                                nc.scalar.dma_start(
                                    bT[:, :],
                                    biasT_in[kc * 128:(kc + 1) * 128, :])
                                nc.scalar.activation(
                                    EB[kc][:, 0:Q], bT[:, :], AF.Exp,
                                    bias=masksb[:, kc:kc + 1], scale=1.0)
                                nc.vector.tensor_copy(EB[kc][:, Q:2 * Q],
                                                      EB[kc][:, 0:Q])

                    # ---- kv AllGather across the batch group ----
                    if tp_k:
                        nc.gpsimd.collective_compute(
                            "AllGather", mybir.AluOpType.bypass,
                            replica_groups=RG,
                            ins=[kbounce[:, :].opt()],
                            outs=[kfull[:, :].opt()])
                    if tp_v:
                        nc.gpsimd.collective_compute(
                            "AllGather", mybir.AluOpType.bypass,
                            replica_groups=RG,
                            ins=[vbounce[:, :].opt()],
                            outs=[vfull[:, :].opt()])

                    # ---- C: attention over head pairs ----
                    with (
                        tc.tile_pool(name="kst", bufs=2) as kstp,
                        tc.tile_pool(name="vph", bufs=2) as vphp,
                        tc.tile_pool(name="eraw", bufs=3) as erawp,
                        tc.tile_pool(name="et", bufs=11) as ep,
                        tc.tile_pool(name="dsb", bufs=1) as dsbp,
                        tc.tile_pool(name="rrep", bufs=1) as rrepp,
                        tc.tile_pool(name="otst", bufs=2) as otstp,
                    ):
                        def load_kst(hp):
                            kst = kstp.tile([128, nkc], bf16, name="kst")
                            nc.sync.dma_start(
                                kst[:, :],
                                kfull[hp * 128:(hp + 1) * 128, :])
                            return kst

                        def load_vquad(hq):
                            vph = vphp.tile([128, KC * 4 * DV], bf16,
                                            name="vph")
                            nc.sync.dma_start(
                                vph[:, :].rearrange(
                                    "p (kc d) -> p kc d", d=4 * DV),
                                vfull[hq * nkc:(hq + 1) * nkc, :]
                                .rearrange("(kc p) d -> p kc d", p=128))
                            return vph

                        def emit_denoms(hp0, psU0):
                            """Softmax normalize + oM write for pair hp0;
                            deferred into the next pair's kc loop so the
                            PE sim stream isn't interrupted."""
                            for sub in range(2):
                                Dsb = dsbp.tile([DV, 512], fr, name="Dsb")
                                nc.vector.tensor_copy(
                                    Dsb[64:65, :], psU0[sub][64:65, :])
                                psR = ps2p.tile([128, 1024], f32,
                                                name="ps2")
                                nc.tensor.matmul(psR[0:64, 0:512],
                                                 ones_r[64:65, 0:64],
                                                 Dsb[64:65, :],
                                                 start=True, stop=True)
                                rrep = rrepp.tile([64, 512], f32,
                                                  name="rrep")
                                nc.vector.reciprocal_approx_fast(
                                    out=rrep[:, :], in_=psR[0:64, 0:512])
                                ot = otstp.tile([64, Q], bf16, name="ot")
                                nc.vector.tensor_mul(ot[:, :],
                                                     psU0[sub][0:64, :],
                                                     rrep[:, :])
                                nc.sync.dma_start(
                                    oM[hp0][sub * 64:(sub + 1) * 64, :],
                                    ot[:, :])

                        kst_cur = load_kst(0)
                        vph_cur = load_vquad(0)
                        prev = None

                        for hp in range(HP):
                            if hp == 6:
                                for i in range(H // 2):
                                    nc.sync.dma_start(
                                        wo[i][:, :],
                                        wo_in[i * 128:(i + 1) * 128, :])
                            if hp % 2 == 1 and hp + 1 < HP:
                                vph_next = load_vquad((hp + 1) // 2)
                            kst_next = (load_kst(hp + 1)
                                        if hp + 1 < HP else None)
                            psU = [psup.tile([DV, 512], f32, name="psu")
                                   for _ in range(2)]
                            pending = []

                            def drain_av(upto):
                                while pending and pending[0][0] <= upto:
                                    kc0, eT = pending.pop(0)
                                    for sub in range(2):
                                        hq_off = (2 * hp + sub) % 4
                                        nc.tensor.matmul(
                                            psU[sub][:, :],
                                            vph_cur[:,
                                                    kc0 * 4 * DV
                                                    + hq_off * DV:
                                                    kc0 * 4 * DV
                                                    + (hq_off + 1) * DV],
                                            eT[:, sub * Q:(sub + 1) * Q],
                                            start=(kc0 == 0),
                                            stop=(kc0 == KC - 1))

                            for kc in range(KC):
                                ps = ps2p.tile([128, 1024], f32,
                                               name="ps2")
                                for sub in range(2):
                                    po = sub * 64
                                    nc.tensor.matmul(
                                        ps[:, sub * Q:(sub + 1) * Q],
                                        kst_cur[po:po + 64,
                                                kc * 128:(kc + 1) * 128],
                                        qT[hp][po:po + 64, :],
                                        start=True, stop=True)
                                if kc == 1 and prev is not None:
                                    emit_denoms(*prev)
                                eRaw = erawp.tile([128, 1024], bf16,
                                                  name="eRaw")
                                nc.scalar.activation(
                                    eRaw[:, :], ps[:, :], AF.Exp,
                                    scale=SCALE)
                                eT = ep.tile([128, 1024], bf16,
                                             name="eT")
                                nc.vector.tensor_mul(eT[:, :], eRaw[:, :],
                                                     EB[kc][:, :])
                                pending.append((kc, eT))
                                # longer lag on hp0 gives the v' AllGather
                                # time to land before the first av needs it
                                lag0 = 8 if mode == "v" else 6
                                drain_av(kc - (lag0 if hp == 0
                                               else AV_LAG))
                            drain_av(KC)
                            prev = (hp, psU)
                            kst_cur = kst_next
                            if hp % 2 == 1 and hp + 1 < HP:
                                vph_cur = vph_next
                        emit_denoms(*prev)

                # ======== stage D ========
                with (
                    tc.tile_pool(name="fin", bufs=3) as finp,
                    tc.tile_pool(name="bop", bufs=1) as bop,
                ):
                    bo_sb = bop.tile([1, F], fr, name="bo_sb")
                    nc.sync.dma_start(bo_sb[:, :], bo_in[:, :])
                    bo_rep = bop.tile([128, F], f32, name="bo_rep")
                    for nb2 in range(2):
                        ps = pskp.tile([128, 512], f32, name="psk")
                        nc.tensor.matmul(ps[:, :], ones_r[0:1, 0:128],
                                         bo_sb[0:1, nb2 * 512:(nb2 + 1) * 512],
                                         start=True, stop=True)
                        nc.vector.tensor_copy(
                            bo_rep[:, nb2 * 512:(nb2 + 1) * 512], ps[:, :])
                    for mc in range(4):
                        for nb2 in range(2):
                            psF = pskp.tile([128, 512], f32, name="psk")
                            for h in range(H // 2):
                                nc.tensor.matmul(
                                    psF[:, :],
                                    oM[h][:, mc * 128:(mc + 1) * 128],
                                    wo[h][:, nb2 * 512:(nb2 + 1) * 512],
                                    start=(h == 0), stop=(h == H // 2 - 1))
                            fin = finp.tile([128, 512], f32, name="fin")
                            nc.vector.tensor_add(
                                fin[:, :], psF[:, :],
                                bo_rep[:, nb2 * 512:(nb2 + 1) * 512])
                            nc.scalar.dma_start(
                                out_t[mc * 128:(mc + 1) * 128,
                                      nb2 * 512:(nb2 + 1) * 512],
                                fin[:, :])

            if niter == 1:
                body()
            else:
                with tc.For_i(0, niter, 1) as iv:
                    body(iv)

    nc.finalize()
    return nc


_nc_cache = {}


def _get_nc(niter=1, nkc=1024, mode="kv"):
    key = (niter, nkc, mode)
    if key not in _nc_cache:
        _nc_cache[key] = build_nc(niter, nkc, mode)
    return _nc_cache[key]


def nkc_for_mask(mask):
    mask = np.asarray(mask)
    nk = int(mask.sum(axis=1).max())
    return max(512, 512 * ((nk + 511) // 512))


def make_in_maps(x, bias, mask, Wq, Wkv, Wo, bo, nkc=None, mode="kv"):
    x = np.asarray(x, dtype=np.float32)
    bias = np.asarray(bias, dtype=np.float32)
    mask = np.asarray(mask)
    if nkc is None:
        nkc = nkc_for_mask(mask)
    KC = nkc // 128
    bfnp = mybir.dt.np(bf16)
    in_maps = []
    wq_h = np.ascontiguousarray(np.asarray(Wq, dtype=np.float32).astype(bfnp))
    wkv_h = np.asarray(Wkv, dtype=np.float32).astype(bfnp)
    wo_h = np.ascontiguousarray(np.asarray(Wo).astype(bfnp))
    bo_h = np.ascontiguousarray(
        np.asarray(bo, dtype=np.float32).reshape(1, F))
    for c in range(8):
        b, r = c // 4, c % 4
        q0 = r * Q
        keys = np.flatnonzero(mask[b])
        nk_eff = len(keys)
        assert nk_eff <= nkc, (nk_eff, nkc)
        xT = np.zeros((F, Q + nkc), dtype=bfnp)
        xT[:, 0:Q] = x[b, q0:q0 + Q].T.astype(bfnp)
        xT[:, Q:Q + nk_eff] = x[b, keys].T.astype(bfnp)
        biasT = np.zeros((nkc, Q), dtype=bfnp)
        biasT[0:nk_eff] = bias[b, q0:q0 + Q][:, keys].T.astype(bfnp)
        maskneg = np.full(nkc, MASK_NEG, dtype=np.float32)
        maskneg[0:nk_eff] = 0.0
        if mode == "kv":
            wk_h = np.ascontiguousarray(wkv_h[:, 256 * r:256 * (r + 1)])
        else:
            wk_h = np.ascontiguousarray(wkv_h[:, 0:F])
        if mode in ("kv", "v"):
            wv_h = np.ascontiguousarray(
                wkv_h[:, F + 256 * r:F + 256 * (r + 1)])
        else:
            wv_h = np.ascontiguousarray(wkv_h[:, F:2 * F])
        in_maps.append({
            "xT_in": np.ascontiguousarray(xT),
            "biasT_in": np.ascontiguousarray(biasT),
            "maskneg_in": np.ascontiguousarray(maskneg.reshape(KC, 128).T),
            "wq_in": wq_h,
            "wk_in": wk_h,
            "wv_in": wv_h,
            "wo_in": wo_h,
            "bo_in": bo_h,
        })
    return in_maps


class _CachedRunner:
    """Jit the NEFF-backed executable once; repeat kernel() calls then skip
    the ~40s relower/recompile and run in ~0.1s."""

    def __init__(self, nc, n_cores=8):
        import jax
        from jax.sharding import Mesh, PartitionSpec
        from jax.experimental.shard_map import shard_map
        from concourse.bass2jax import (_bass_exec_p, install_neuronx_cc_hook,
                                        partition_id_tensor)
        install_neuronx_cc_hook()
        self.jax = jax
        self.n_cores = n_cores
        pname = nc.partition_id_tensor.name if nc.partition_id_tensor else None
        in_names, out_names, out_avals, zeros = [], [], [], []
        for alloc in nc.m.functions[0].allocations:
            if not isinstance(alloc, mybir.MemoryLocationSet):
                continue
            name = alloc.memorylocations[0].name
            if alloc.kind == "ExternalInput":
                if name != pname:
                    in_names.append(name)
            elif alloc.kind == "ExternalOutput":
                out_names.append(name)
                shape = tuple(alloc.tensor_shape)
                dt_np = mybir.dt.np(alloc.dtype)
                out_avals.append(jax.core.ShapedArray(shape, dt_np))
                zeros.append(np.zeros(shape, dt_np))
        self.in_names, self.out_names = in_names, out_names
        self.out_avals, self.zeros = out_avals, zeros
        all_names = in_names + out_names + ([pname] if pname else [])

        def _body(*args):
            ops = list(args)
            if pname is not None:
                ops.append(partition_id_tensor())
            return tuple(_bass_exec_p.bind(
                *ops, out_avals=tuple(out_avals), in_names=tuple(all_names),
                out_names=tuple(out_names), lowering_input_output_aliases=(),
                sim_require_finite=True, sim_require_nnan=True, nc=nc))

        mesh = Mesh(np.asarray(jax.devices()[:n_cores]), ("core",))
        spec_in = (PartitionSpec("core"),) * (len(in_names) + len(out_names))
        spec_out = (PartitionSpec("core"),) * len(out_names)
        self.fn = jax.jit(shard_map(_body, mesh=mesh, in_specs=spec_in,
                                    out_specs=spec_out, check_rep=False),
                          keep_unused=True)

    def run(self, in_maps):
        n = self.n_cores
        args = [np.concatenate([np.asarray(in_maps[c][k]) for c in range(n)], axis=0)
                for k in self.in_names]
        args += [np.zeros((n * z.shape[0], *z.shape[1:]), z.dtype)
                 for z in self.zeros]
        outs = self.fn(*args)
        self.jax.block_until_ready(outs)
        return [{k: np.asarray(outs[i]).reshape(n, *self.out_avals[i].shape)[c]
                 for i, k in enumerate(self.out_names)} for c in range(n)]


_runner_cache = {}


MODE = "none"


def kernel(x, bias, mask, Wq, Wkv, Wo, bo):
    nkc = nkc_for_mask(mask)
    try:
        key = (nkc, MODE)
        if key not in _runner_cache:
            _runner_cache[key] = _CachedRunner(_get_nc(1, nkc, MODE))
        in_maps = make_in_maps(x, bias, mask, Wq, Wkv, Wo, bo, nkc=nkc,
                               mode=MODE)
        results = _runner_cache[key].run(in_maps)
    except Exception:
        # robust fallback: no-collective variant (k/v computed locally)
        _runner_cache.pop((nkc, MODE), None)
        in_maps = make_in_maps(x, bias, mask, Wq, Wkv, Wo, bo, nkc=nkc,
                               mode="none")
        res = run_bass_kernel_spmd(_get_nc(1, nkc, "none"), in_maps,
                                   core_ids=list(range(8)))
        results = res.results
    out = np.empty((2, 2048, F), dtype=np.float32)
    for c in range(8):
        b, qi = c // 4, c % 4
        out[b, qi * Q:(qi + 1) * Q] = results[c]["out_t"]
    return out


# revision 30
# speedup vs baseline: 7.7062x; 1.3083x over previous
"""Trainium2 Bass kernel for nn_Attention_88441966559243.

Attention with additive bias [B,N,N] and per-key bool mask, fp32.
  B=2, N=2048, QD=1024, HEADS=16, DIM_HEAD=64.

Sharding: 8 cores = (batch b = core//4) x (query slice q0 = (core%4)*512).
Each core computes out[b, q0:q0+512, :] completely on-device; the host
gather is a pure concatenation.  MODE="none": no collectives (AllGather kv
tensor-parallel modes "kv"/"v" are implemented but measured a net loss on
this runtime: ~+400us per exec for the collectives vs ~31us of saved PE).

Key compaction: masked keys contribute exactly zero to the softmax
(exp(bias - 30000) underflows to 0, matching the reference's exp(-FMAX)),
so the host gathers only the unmasked keys per batch and pads to NKC
(multiple of 512; 1024 for the ~50% graded mask).  This halves key-side
projections, sim, av, the ACT exp stream and most of the DMA.  Padded key
slots get maskneg=-30000 so EB=0 and they contribute nothing (numerator or
denominator).

All inputs ride in bf16 (x, bias, weights); projections accumulate in fp32
PSUM; e/v'/k/q and the final out store are bf16 (host upcasts to f32).
Measured rel err vs the fp32 reference 7.1e-3 (gate 2e-2).

Per-core schedule (single pass):
  - PE p-state warmup matmuls ramp the tensor engine during the DMA prefix.
  - A2 q-projection first (qT bf16), then k-pair-0 slab (SBUF-resident) and
    v'-quad-0, so C starts as early as possible; EB = exp(biasT + maskneg)
    rides the ACT queue meanwhile.
  - C: per head-pair hp: sim = kst^T q as paired K=64 matmuls;
    e = exp(sim*scale)*EB bf16 (ACT exp, DVE mul); av accumulates [v|1]^T e
    into PSUM (the ones column yields the softmax denominator for free);
    normalize+oM write deferred into the next pair's loop.  The remaining
    k-pair slabs and v' quads are STREAMED through the hp prologues, so
    stage A's PE tail hides under C's ACT-gated slots.
  - D: out = ot^T Wo + bo as K=128 bf16 matmuls.

Measured on HW (8 cores, For_i steady state, axon wall-delta method):
152-178us/iter across environment phases (best pair-median 151.8us) vs
377us for the handoff baseline measured the same way (absolute numbers
drift; relative gain ~2.2-2.5x).
Cost-model sim: ~180us single-shot, PE 147.8us busy (80%), DVE 108.6us,
ACT 72.9us; C is ACT/chain-gated at ~1.1us per (hp,kc) slot.
"""
import sys
for _p in ("/opt/trn_rl_repo", "/root/.axon_site/_ro/trn_rl_repo"):
    if _p not in sys.path:
        sys.path.insert(0, _p)

import numpy as np

import concourse.bass as bass
import concourse.mybir as mybir
from concourse import bacc
from concourse.tile import TileContext
from concourse.bass_utils import run_bass_kernel_spmd

F = 1024          # feature dim (QD == INNER)
Q = 512           # queries per core
H = 16            # heads
D = 64            # head dim
DV = 65           # head dim + ones column
SCALE = D ** -0.5
MASK_NEG = -30000.0

FC = F // 128      # 8 feature chunks
HP = H // 2        # 8 head pairs

f32 = mybir.dt.float32
fr = mybir.dt.float32r
bf16 = mybir.dt.bfloat16
AF = mybir.ActivationFunctionType

AV_LAG = 3         # av matmuls trail the exp/mul producers by this many kc
RG = [[0, 1, 2, 3], [4, 5, 6, 7]]   # batch groups for the kv AllGather


def build_nc(niter: int = 1, nkc: int = 1024, mode: str = "kv"):
    KC = nkc // 128    # key chunks
    NB = nkc // 512    # key 512-blocks
    assert nkc % 512 == 0 and nkc >= 512
    assert mode in ("kv", "v", "none")
    # mode "kv": this core projects k for 2 head-pairs and v for 4 heads,
    #   and the batch group AllGathers both (2 collectives).
    # mode "v": k computed fully locally; only the v' quad is AllGathered
    #   (1 collective, hidden behind a deep hp0 av lag).
    # mode "none": all local, no collectives (robust fallback).
    tp_k = mode == "kv"
    tp_v = mode in ("kv", "v")
    KWK = 256 if tp_k else 1024   # wk input columns
    KWV = 256 if tp_v else 1024   # wv input columns

    nc = bacc.Bacc(None, target_bir_lowering=False, num_devices=8)

    xT_in = nc.dram_tensor("xT_in", [F, Q + nkc], bf16, kind="ExternalInput")
    biasT_in = nc.dram_tensor("biasT_in", [nkc, Q], bf16,
                              kind="ExternalInput")
    maskneg_in = nc.dram_tensor("maskneg_in", [128, KC], f32,
                                kind="ExternalInput")
    wq_in = nc.dram_tensor("wq_in", [F, F], bf16, kind="ExternalInput")
    wk_in = nc.dram_tensor("wk_in", [F, KWK], bf16, kind="ExternalInput")
    wv_in = nc.dram_tensor("wv_in", [F, KWV], bf16, kind="ExternalInput")
    wo_in = nc.dram_tensor("wo_in", [F, F], bf16, kind="ExternalInput")
    bo_in = nc.dram_tensor("bo_in", [1, F], fr, kind="ExternalInput")
    out_t = nc.dram_tensor("out_t", [Q, F], f32, kind="ExternalOutput")

    with TileContext(nc) as tc:
        with (
            tc.tile_pool(name="const", bufs=1) as constp,
            tc.tile_pool(name="dram", bufs=1, space="DRAM") as dramp,
            tc.tile_pool(name="ps2", bufs=2, space="PSUM") as ps2p,   # [128,1024]
            tc.tile_pool(name="psk", bufs=2, space="PSUM") as pskp,   # [128,512]
            tc.tile_pool(name="psu", bufs=2, space="PSUM") as psup,   # [DV,512]
        ):
            ones_f = constp.tile([128, 128], f32)
            nc.vector.memset(ones_f[:, :], 1.0)
            ones_r = constp.tile([128, 128], fr)
            nc.scalar.copy(ones_r[:, :], ones_f[:, :])
            masksb = constp.tile([128, KC], f32)
            nc.sync.dma_start(masksb[:, :], maskneg_in[:, :])

            kfull = dramp.tile([8 * 128, nkc], bf16)    # all 8 pair slabs
            vfull = dramp.tile([4 * nkc, 4 * DV], bf16)  # all 4 quads
            if tp_k:
                kbounce = dramp.tile([256, nkc], bf16)   # my 2 k-pair slabs
            if tp_v:
                vbounce = dramp.tile([nkc, 4 * DV], bf16)  # my v' quad

            def body(_iv=None):
              with (
                  tc.tile_pool(name="oMp", bufs=1) as oMp,
                  tc.tile_pool(name="wop", bufs=1) as wop,
              ):
                # out^T head-pair tiles (bf16), written in-place by C's ot
                # DMAs; wo (bf16) loaded near the end of C.  Stacking heads
                # 2i/2i+1 on partitions 0:64/64:128 makes D K=128 full-rate.
                oM = [oMp.tile([128, Q], bf16, tag=f"oM{i}", name=f"oM{i}")
                      for i in range(H // 2)]
                wo = [wop.tile([128, F], bf16, tag=f"wo{i}", name=f"wo{i}")
                      for i in range(H // 2)]
                with (
                    tc.tile_pool(name="EBp", bufs=1) as EBp,
                    tc.tile_pool(name="qTp", bufs=1) as qTp,
                ):
                    EB = [EBp.tile([128, 2 * Q], bf16, tag=f"EB{i}",
                                   name=f"EB{i}") for i in range(KC)]
                    qT = [qTp.tile([128, Q], bf16, tag=f"qT{i}",
                                   name=f"qT{i}") for i in range(FC)]

                    # ---- A: per-core K/V quarter projections + q + EB ----
                    with (
                        tc.tile_pool(name="xTp", bufs=1) as xTp,
                        tc.tile_pool(name="wload", bufs=8) as wlp,
                        tc.tile_pool(name="wkv", bufs=1) as wkvp,
                        tc.tile_pool(name="kpsb", bufs=2) as kpsbp,
                        tc.tile_pool(name="vstg", bufs=6) as vstgp,
                    ):
                        xTc = [xTp.tile([128, 512], bf16, tag=f"xT{i}",
                                        name=f"xT{i}")
                               for i in range((1 + NB) * FC)]

                        def xt(fc, cb):
                            return xTc[fc * (1 + NB) + cb]

                        # PE p-state warmup: ~3-4us of junk matmuls on the
                        # ones tile ramp the tensor engine to full clock
                        # while the input DMAs land, so stage A's real
                        # matmuls start at 2.4GHz instead of 0.65-1.2GHz.
                        ps_w = pskp.tile([128, 512], f32, name="psk")
                        for _w in range(24):
                            nc.tensor.matmul(
                                ps_w[:, 0:128], ones_r[:, :], ones_r[:, :],
                                start=(_w == 0), stop=(_w == 23))
                        warm_sink = constp.tile([1, 128], f32)
                        nc.vector.tensor_copy(warm_sink[:, :],
                                              ps_w[0:1, 0:128])

                        # loads: wk/wv/xk first (k/v parts fire the
                        # collectives), then wq/xq for A2.
                        wk_sb = wkvp.tile([128, FC * KWK], bf16, tag="wk",
                                          name="wk")
                        nc.sync.dma_start(
                            wk_sb[:, :].rearrange("p (fc c) -> p fc c",
                                                  c=KWK),
                            wk_in[0:F, :].rearrange("(fc p) c -> p fc c",
                                                    p=128))
                        wv_sb = wkvp.tile([128, FC * KWV], bf16, tag="wv",
                                          name="wv")
                        nc.sync.dma_start(
                            wv_sb[:, :].rearrange("p (fc c) -> p fc c",
                                                  c=KWV),
                            wv_in[0:F, :].rearrange("(fc p) c -> p fc c",
                                                    p=128))
                        for cb in range(1, 1 + NB):
                            for fc in range(FC):
                                nc.sync.dma_start(
                                    xt(fc, cb)[:, :],
                                    xT_in[fc * 128:(fc + 1) * 128,
                                          Q + (cb - 1) * 512:Q + cb * 512])
                        wq = [wlp.tile([128, F], bf16, tag="w", name="w")
                              for _ in range(FC)]
                        for fc in range(FC):
                            nc.sync.dma_start(
                                wq[fc][:, :],
                                wq_in[fc * 128:(fc + 1) * 128, :])
                        for fc in range(FC):
                            nc.sync.dma_start(
                                xt(fc, 0)[:, :],
                                xT_in[fc * 128:(fc + 1) * 128, 0:512])

                        # A-k: one head-pair k slab; mode "none"
                        # keeps it SBUF-resident and streams pairs 1-7
                        # through C's hp prologues (overlapping A's PE
                        # tail under C's ACT-gated slots).
                        def emit_kpair(pair):
                            kp_sb = kpsbp.tile([128, nkc], bf16,
                                               name="kpsb")
                            for nb in range(NB):
                                ps = pskp.tile([128, 512], f32, name="psk")
                                for fc in range(FC):
                                    nc.tensor.matmul(
                                        ps[:, :],
                                        wk_sb[:, :].rearrange(
                                            "p (fc c) -> p fc c",
                                            c=KWK)[:, fc,
                                                  pair * 128:
                                                  (pair + 1) * 128],
                                        xt(fc, 1 + nb)[:, :],
                                        start=(fc == 0),
                                        stop=(fc == FC - 1))
                                nc.vector.tensor_copy(
                                    kp_sb[:, nb * 512:(nb + 1) * 512],
                                    ps[:, :])
                            if tp_k:
                                nc.sync.dma_start(
                                    kbounce[pair * 128:(pair + 1) * 128,
                                            :],
                                    kp_sb[:, :])
                            return kp_sb

                        # A-v: one v' quad; mode "none" streams
                        # quads 1-3 through C's hp prologues
                        def emit_vquad(hq):
                            for kc in range(KC):
                                vst = vstgp.tile([128, 4 * DV], bf16,
                                                 name="vst")
                                nc.vector.memset(
                                    vst[:, :].rearrange(
                                        "p (h x) -> p h x",
                                        x=DV)[:, :, D:DV], 1.0)
                                ps = ps2p.tile([128, 1024], f32,
                                               name="ps2")
                                for fc in range(FC):
                                    nc.tensor.matmul(
                                        ps[:, 0:256],
                                        xt(fc, 1 + kc // 4)[
                                            :, (kc % 4) * 128:
                                            (kc % 4 + 1) * 128],
                                        wv_sb[:, fc * KWV + hq * 256:
                                              fc * KWV + (hq + 1) * 256],
                                        start=(fc == 0),
                                        stop=(fc == FC - 1))
                                nc.vector.tensor_copy(
                                    vst[:, :].rearrange(
                                        "p (h x) -> p h x",
                                        x=DV)[:, :, 0:D],
                                    ps[:, 0:256].rearrange(
                                        "p (h d) -> p h d", d=D))
                                if tp_v:
                                    nc.sync.dma_start(
                                        vbounce[kc * 128:(kc + 1) * 128,
                                                :],
                                        vst[:, :])
                                else:
                                    nc.sync.dma_start(
                                        vfull[hq * nkc + kc * 128:
                                              hq * nkc + (kc + 1) * 128,
                                              :],
                                        vst[:, :])

                        # A2: qT[m] = Wq[:,m]^T @ xT[:, 0:Q]  (unscaled;
                        # SCALE rides on the exp activation's scale).
                        # Emitted FIRST so C's sims can start as soon as
                        # pair 0's slab is ready.
                        for m in range(FC):
                            ps = pskp.tile([128, 512], f32, name="psk")
                            for fc in range(FC):
                                nc.tensor.matmul(
                                    ps[:, :],
                                    wq[fc][:, m * 128:(m + 1) * 128],
                                    xt(fc, 0)[:, :],
                                    start=(fc == 0), stop=(fc == FC - 1))
                            nc.vector.tensor_copy(qT[m][:, :], ps[:, :])

                        kst_sb = {}
                        if mode == "none":
                            kst_sb[0] = emit_kpair(0)
                            emit_vquad(0)
                        else:
                            for pair in range(2 if tp_k else 8):
                                kst_sb[pair] = emit_kpair(pair)
                            for hq in range(1 if tp_v else 4):
                                emit_vquad(hq)

                        # B: EB = exp(biasT + maskneg); biasT loads ride
                        # the ACT hwdge queue
                        with tc.tile_pool(name="bT", bufs=4) as bTp:
                            for kc in range(KC):
                                bT = bTp.tile([128, Q], bf16, name="bT")
                                nc.scalar.dma_start(
                                    bT[:, :],
                                    biasT_in[kc * 128:(kc + 1) * 128, :])
                                nc.scalar.activation(
                                    EB[kc][:, 0:Q], bT[:, :], AF.Exp,
                                    bias=masksb[:, kc:kc + 1], scale=1.0)
                                nc.vector.tensor_copy(EB[kc][:, Q:2 * Q],
                                                      EB[kc][:, 0:Q])

                    # ---- kv AllGather across the batch group ----
                    if tp_k:
                        nc.gpsimd.collective_compute(
                            "AllGather", mybir.AluOpType.bypass,
                            replica_groups=RG,
                            ins=[kbounce[:, :].opt()],
                            outs=[kfull[:, :].opt()])
                    if tp_v:
                        nc.gpsimd.collective_compute(
                            "AllGather", mybir.AluOpType.bypass,
                            replica_groups=RG,
                            ins=[vbounce[:, :].opt()],
                            outs=[vfull[:, :].opt()])

                    # ---- C: attention over head pairs ----
                    with (
                        tc.tile_pool(name="kst", bufs=2) as kstp,
                        tc.tile_pool(name="vph", bufs=2) as vphp,
                        tc.tile_pool(name="eraw", bufs=3) as erawp,
                        tc.tile_pool(name="et", bufs=11) as ep,
                        tc.tile_pool(name="dsb", bufs=1) as dsbp,
                        tc.tile_pool(name="rrep", bufs=1) as rrepp,
                        tc.tile_pool(name="otst", bufs=2) as otstp,
                    ):
                        def load_kst(hp):
                            kst = kstp.tile([128, nkc], bf16, name="kst")
                            nc.sync.dma_start(
                                kst[:, :],
                                kfull[hp * 128:(hp + 1) * 128, :])
                            return kst

                        def load_vquad(hq):
                            vph = vphp.tile([128, KC * 4 * DV], bf16,
                                            name="vph")
                            nc.sync.dma_start(
                                vph[:, :].rearrange(
                                    "p (kc d) -> p kc d", d=4 * DV),
                                vfull[hq * nkc:(hq + 1) * nkc, :]
                                .rearrange("(kc p) d -> p kc d", p=128))
                            return vph

                        def emit_denoms(hp0, psU0):
                            """Softmax normalize + oM write for pair hp0;
                            deferred into the next pair's kc loop so the
                            PE sim stream isn't interrupted."""
                            for sub in range(2):
                                Dsb = dsbp.tile([DV, 512], fr, name="Dsb")
                                nc.vector.tensor_copy(
                                    Dsb[64:65, :], psU0[sub][64:65, :])
                                psR = ps2p.tile([128, 1024], f32,
                                                name="ps2")
                                nc.tensor.matmul(psR[0:64, 0:512],
                                                 ones_r[64:65, 0:64],
                                                 Dsb[64:65, :],
                                                 start=True, stop=True)
                                rrep = rrepp.tile([64, 512], f32,
                                                  name="rrep")
                                nc.vector.reciprocal_approx_fast(
                                    out=rrep[:, :], in_=psR[0:64, 0:512])
                                ot = otstp.tile([64, Q], bf16, name="ot")
                                nc.vector.tensor_mul(ot[:, :],
                                                     psU0[sub][0:64, :],
                                                     rrep[:, :])
                                nc.sync.dma_start(
                                    oM[hp0][sub * 64:(sub + 1) * 64, :],
                                    ot[:, :])

                        kst_cur = load_kst(0)
                        vph_cur = load_vquad(0)
                        prev = None

                        for hp in range(HP):
                            if hp == 6:
                                for i in range(H // 2):
                                    nc.sync.dma_start(
                                        wo[i][:, :],
                                        wo_in[i * 128:(i + 1) * 128, :])
                            if hp % 2 == 1 and hp + 1 < HP:
                                vph_next = load_vquad((hp + 1) // 2)
                            kst_next = (load_kst(hp + 1)
                                        if hp + 1 < HP else None)
                            psU = [psup.tile([DV, 512], f32, name="psu")
                                   for _ in range(2)]
                            pending = []

                            def drain_av(upto):
                                while pending and pending[0][0] <= upto:
                                    kc0, eT = pending.pop(0)
                                    for sub in range(2):
                                        hq_off = (2 * hp + sub) % 4
                                        nc.tensor.matmul(
                                            psU[sub][:, :],
                                            vph_cur[:,
                                                    kc0 * 4 * DV
                                                    + hq_off * DV:
                                                    kc0 * 4 * DV
                                                    + (hq_off + 1) * DV],
                                            eT[:, sub * Q:(sub + 1) * Q],
                                            start=(kc0 == 0),
                                            stop=(kc0 == KC - 1))

                            for kc in range(KC):
                                ps = ps2p.tile([128, 1024], f32,
                                               name="ps2")
                                for sub in range(2):
                                    po = sub * 64
                                    nc.tensor.matmul(
                                        ps[:, sub * Q:(sub + 1) * Q],
                                        kst_cur[po:po + 64,
                                                kc * 128:(kc + 1) * 128],
                                        qT[hp][po:po + 64, :],
                                        start=True, stop=True)
                                if kc == 1 and prev is not None:
                                    emit_denoms(*prev)
                                eRaw = erawp.tile([128, 1024], bf16,
                                                  name="eRaw")
                                nc.scalar.activation(
                                    eRaw[:, :], ps[:, :], AF.Exp,
                                    scale=SCALE)
                                eT = ep.tile([128, 1024], bf16,
                                             name="eT")
                                nc.vector.tensor_mul(eT[:, :], eRaw[:, :],
                                                     EB[kc][:, :])
                                pending.append((kc, eT))
                                # longer lag on hp0 gives the v' AllGather
                                # time to land before the first av needs it
                                lag0 = 8 if mode == "v" else 6
                                drain_av(kc - (lag0 if hp == 0
                                               else AV_LAG))
                            drain_av(KC)
                            prev = (hp, psU)
                            kst_cur = kst_next
                            if hp % 2 == 1 and hp + 1 < HP:
                                vph_cur = vph_next
                        emit_denoms(*prev)

                # ======== stage D ========
                with (
                    tc.tile_pool(name="fin", bufs=3) as finp,
                    tc.tile_pool(name="bop", bufs=1) as bop,
                ):
                    bo_sb = bop.tile([1, F], fr, name="bo_sb")
                    nc.sync.dma_start(bo_sb[:, :], bo_in[:, :])
                    bo_rep = bop.tile([128, F], f32, name="bo_rep")
                    for nb2 in range(2):
                        ps = pskp.tile([128, 512], f32, name="psk")
                        nc.tensor.matmul(ps[:, :], ones_r[0:1, 0:128],
                                         bo_sb[0:1, nb2 * 512:(nb2 + 1) * 512],
                                         start=True, stop=True)
                        nc.vector.tensor_copy(
                            bo_rep[:, nb2 * 512:(nb2 + 1) * 512], ps[:, :])
                    for mc in range(4):
                        for nb2 in range(2):
                            psF = pskp.tile([128, 512], f32, name="psk")
                            for h in range(H // 2):
                                nc.tensor.matmul(
                                    psF[:, :],
                                    oM[h][:, mc * 128:(mc + 1) * 128],
                                    wo[h][:, nb2 * 512:(nb2 + 1) * 512],
                                    start=(h == 0), stop=(h == H // 2 - 1))
                            fin = finp.tile([128, 512], f32, name="fin")
                            nc.vector.tensor_add(
                                fin[:, :], psF[:, :],
                                bo_rep[:, nb2 * 512:(nb2 + 1) * 512])
                            nc.scalar.dma_start(
                                out_t[mc * 128:(mc + 1) * 128,
                                      nb2 * 512:(nb2 + 1) * 512],
                                fin[:, :])

            if niter == 1:
                body()
            else:
                with tc.For_i(0, niter, 1) as iv:
                    body(iv)

    nc.finalize()
    return nc


_nc_cache = {}


def _get_nc(niter=1, nkc=1024, mode="kv"):
    key = (niter, nkc, mode)
    if key not in _nc_cache:
        _nc_cache[key] = build_nc(niter, nkc, mode)
    return _nc_cache[key]


def nkc_for_mask(mask):
    mask = np.asarray(mask)
    nk = int(mask.sum(axis=1).max())
    return max(512, 512 * ((nk + 511) // 512))


def make_in_maps(x, bias, mask, Wq, Wkv, Wo, bo, nkc=None, mode="kv"):
    x = np.asarray(x, dtype=np.float32)
    bias = np.asarray(bias, dtype=np.float32)
    mask = np.asarray(mask)
    if nkc is None:
        nkc = nkc_for_mask(mask)
    KC = nkc // 128
    bfnp = mybir.dt.np(bf16)
    in_maps = []
    wq_h = np.ascontiguousarray(np.asarray(Wq, dtype=np.float32).astype(bfnp))
    wkv_h = np.asarray(Wkv, dtype=np.float32).astype(bfnp)
    wo_h = np.ascontiguousarray(np.asarray(Wo).astype(bfnp))
    bo_h = np.ascontiguousarray(
        np.asarray(bo, dtype=np.float32).reshape(1, F))
    for c in range(8):
        b, r = c // 4, c % 4
        q0 = r * Q
        keys = np.flatnonzero(mask[b])
        nk_eff = len(keys)
        assert nk_eff <= nkc, (nk_eff, nkc)
        xT = np.zeros((F, Q + nkc), dtype=bfnp)
        xT[:, 0:Q] = x[b, q0:q0 + Q].T.astype(bfnp)
        xT[:, Q:Q + nk_eff] = x[b, keys].T.astype(bfnp)
        biasT = np.zeros((nkc, Q), dtype=bfnp)
        biasT[0:nk_eff] = bias[b, q0:q0 + Q][:, keys].T.astype(bfnp)
        maskneg = np.full(nkc, MASK_NEG, dtype=np.float32)
        maskneg[0:nk_eff] = 0.0
        if mode == "kv":
            wk_h = np.ascontiguousarray(wkv_h[:, 256 * r:256 * (r + 1)])
        else:
            wk_h = np.ascontiguousarray(wkv_h[:, 0:F])
        if mode in ("kv", "v"):
            wv_h = np.ascontiguousarray(
                wkv_h[:, F + 256 * r:F + 256 * (r + 1)])
        else:
            wv_h = np.ascontiguousarray(wkv_h[:, F:2 * F])
        in_maps.append({
            "xT_in": np.ascontiguousarray(xT),
            "biasT_in": np.ascontiguousarray(biasT),
            "maskneg_in": np.ascontiguousarray(maskneg.reshape(KC, 128).T),
            "wq_in": wq_h,
            "wk_in": wk_h,
            "wv_in": wv_h,
            "wo_in": wo_h,
            "bo_in": bo_h,
        })
    return in_maps


class _CachedRunner:
    """Jit the NEFF-backed executable once; repeat kernel() calls then skip
    the ~40s relower/recompile and run in ~0.1s."""

    def __init__(self, nc, n_cores=8):
        import jax
        from jax.sharding import Mesh, PartitionSpec
        from jax.experimental.shard_map import shard_map
        from concourse.bass2jax import (_bass_exec_p, install_neuronx_cc_hook,
                                        partition_id_tensor)
        install_neuronx_cc_hook()
        self.jax = jax
        self.n_cores = n_cores
        pname = nc.partition_id_tensor.name if nc.partition_id_tensor else None
        in_names, out_names, out_avals, zeros = [], [], [], []
        for alloc in nc.m.functions[0].allocations:
            if not isinstance(alloc, mybir.MemoryLocationSet):
                continue
            name = alloc.memorylocations[0].name
            if alloc.kind == "ExternalInput":
                if name != pname:
                    in_names.append(name)
            elif alloc.kind == "ExternalOutput":
                out_names.append(name)
                shape = tuple(alloc.tensor_shape)
                dt_np = mybir.dt.np(alloc.dtype)
                out_avals.append(jax.core.ShapedArray(shape, dt_np))
                zeros.append(np.zeros(shape, dt_np))
        self.in_names, self.out_names = in_names, out_names
        self.out_avals, self.zeros = out_avals, zeros
        all_names = in_names + out_names + ([pname] if pname else [])

        def _body(*args):
            ops = list(args)
            if pname is not None:
                ops.append(partition_id_tensor())
            return tuple(_bass_exec_p.bind(
                *ops, out_avals=tuple(out_avals), in_names=tuple(all_names),
                out_names=tuple(out_names), lowering_input_output_aliases=(),
                sim_require_finite=True, sim_require_nnan=True, nc=nc))

        mesh = Mesh(np.asarray(jax.devices()[:n_cores]), ("core",))
        spec_in = (PartitionSpec("core"),) * (len(in_names) + len(out_names))
        spec_out = (PartitionSpec("core"),) * len(out_names)
        self.fn = jax.jit(shard_map(_body, mesh=mesh, in_specs=spec_in,
                                    out_specs=spec_out, check_rep=False),
                          keep_unused=True)

    def run(self, in_maps):
        n = self.n_cores
        args = [np.concatenate([np.asarray(in_maps[c][k]) for c in range(n)], axis=0)
                for k in self.in_names]
        args += [np.zeros((n * z.shape[0], *z.shape[1:]), z.dtype)
                 for z in self.zeros]
        outs = self.fn(*args)
        self.jax.block_until_ready(outs)
        return [{k: np.asarray(outs[i]).reshape(n, *self.out_avals[i].shape)[c]
                 for i, k in enumerate(self.out_names)} for c in range(n)]


_runner_cache = {}


MODE = "none"


def kernel(x, bias, mask, Wq, Wkv, Wo, bo):
    nkc = nkc_for_mask(mask)
    try:
        key = (nkc, MODE)
        if key not in _runner_cache:
            _runner_cache[key] = _CachedRunner(_get_nc(1, nkc, MODE))
        in_maps = make_in_maps(x, bias, mask, Wq, Wkv, Wo, bo, nkc=nkc,
                               mode=MODE)
        results = _runner_cache[key].run(in_maps)
    except Exception:
        # robust fallback: no-collective variant (k/v computed locally)
        _runner_cache.pop((nkc, MODE), None)
        in_maps = make_in_maps(x, bias, mask, Wq, Wkv, Wo, bo, nkc=nkc,
                               mode="none")
        res = run_bass_kernel_spmd(_get_nc(1, nkc, "none"), in_maps,
                                   core_ids=list(range(8)))
        results = res.results
    out = np.empty((2, 2048, F), dtype=np.float32)
    for c in range(8):
        b, qi = c // 4, c % 4
        out[b, qi * Q:(qi + 1) * Q] = results[c]["out_t"]
    return out
